# revision 12
# baseline (speedup 1.0000x reference)
"""Trainium2 Bass kernel for DiceLoss (hard-argmax dice, ignore background, mean).

Problem (hardcoded shapes):
  y_true: [16, 512, 512] int32 in [0, 8)
  y_pred: [16, 8, 512, 512] float32
  out   : scalar float32 = mean over classes 1..7 of
          (2*tp + eps) / (2*tp + fp + fn + eps)
  with pred_cls = argmax_c y_pred, one-hot tp/fp/fn sums over all pixels.

Strategy (8 NeuronCores, data-parallel over batch; 2 images per core):
  - Each channel plane is one [128, 2048] tile. y_pred is loaded via SWDGE
    cast-DMA (f32 in HBM -> bf16 in SBUF): HBM read traffic is unchanged but
    every on-chip elementwise op runs in DVE 16-bit perf modes and no
    convert instructions are needed.
  - DVE (all bf16, no accum_out so the 2x/4x perf-mode uops stay eligible):
      * 7-op pairwise max tree -> m = max over channels      (2x_1P)
      * pred_c = (ch_c == m) via tensor_tensor is_equal      (2x_1P)
      * gt_c   = (tf == c) via tensor_single_scalar is_equal (4x_2P),
        written strided into a [128, 16, 130] block layout whose col 128
        holds a persistent ones column (memset once).
  - ScalarE: int32->bf16 label convert; per class a copy-with-accum_out of
    gt_c that yields the per-partition gt counts; PSUM evacuation.
  - TensorE: per class-subtile one matmul with lhsT = pred subtile and
    rhs = [gt subtile | ones] (129 cols) accumulated over subtiles+images:
    diag gives tp, column 128 gives pred counts. Host reads trace + sums.
  - Host: combines the 8 cores' exact-integer f32 partials and forms the
    dice mean in float32, matching the reference arithmetic.
"""

import numpy as np

EPS = 1e-05

# Problem geometry (hardcoded per the harness contract).
N_CORES = 8
NB = 2            # batch images per core
C = 8             # classes
P = 128           # SBUF partitions
FD = 2048         # free-dim elements per channel plane (512*512 = 128*2048)
NSUB = FD // 128  # 128-wide subtiles per plane for the PE matmuls
BLK = 130         # gt block stride: 128 gt cols + ones col + 1 pad (4B align)

_CACHED_NC = None


def build_bass():
    """Build the Bass kernel (same NEFF for all 8 cores)."""
    from contextlib import ExitStack

    import concourse.bacc as bacc
    import concourse.tile as tile
    from concourse import mybir

    nc = bacc.Bacc(None, target_bir_lowering=False)

    yp = nc.dram_tensor("yp", [NB, C, P, FD], mybir.dt.float32, kind="ExternalInput")
    yt = nc.dram_tensor("yt", [NB, P, FD], mybir.dt.int32, kind="ExternalInput")
    # per class: [128, 129] PSUM accumulator (cross-products + pred colsum).
    mm_out = nc.dram_tensor("mm_out", [7, P, 129], mybir.dt.float32, kind="ExternalOutput")
    # per-partition gt counts: slots 0..6 = (img0, class), 7..13 = (img1,
    # half 0, class), 14..20 = (img1, half 1, class)
    ga_out = nc.dram_tensor("ga_out", [P, 21], mybir.dt.float32, kind="ExternalOutput")

    with tile.TileContext(nc) as tc, ExitStack() as ctx:
        chpool = ctx.enter_context(tc.tile_pool(name="ch", bufs=1))
        tpool = ctx.enter_context(tc.tile_pool(name="tt", bufs=1))
        mpool = ctx.enter_context(tc.tile_pool(name="mx", bufs=2))
        mtmp = ctx.enter_context(tc.tile_pool(name="mtmp", bufs=6))
        predp = ctx.enter_context(tc.tile_pool(name="pred", bufs=6))
        scrp = ctx.enter_context(tc.tile_pool(name="scr", bufs=2))
        accp = ctx.enter_context(tc.tile_pool(name="acc", bufs=1))
        psump = ctx.enter_context(tc.tile_pool(name="psum", bufs=1, space="PSUM"))

        ga_acc = accp.tile([P, 21], mybir.dt.float32, name="ga_acc")
        # fixed per-class gt tiles in block layout [128, 16, 130]:
        # cols 0:128 = gt mask (rewritten per image), col 128 = ones,
        # col 129 = zero pad (so a flat [128, 2080] read sums cleanly).
        gts = [
            accp.tile([P, NSUB, BLK], mybir.dt.bfloat16, name=f"gt{c}")
            for c in range(1, C)
        ]
        psums = [
            psump.tile([P, 129], mybir.dt.float32, name=f"ps{c}", tag=f"ps{c}")
            for c in range(1, C)
        ]

        for g in gts:
            nc.vector.memset(g[:, :, 128:129], 1.0)
            nc.vector.memset(g[:, :, 129:130], 0.0)

        HF = FD // 2   # half-plane free dim
        HS = NSUB // 2  # subtiles per half

        # ---- all loads up front: gpsimd queue delivers the casts FIFO in
        # exactly this order; labels ride the concurrent HWDGE queue.
        # Image 0 loads whole planes; image 1 loads half-planes (half-major)
        # so the tail after the last byte is only half a plane's compute. ----
        ch = {}   # (n, c) -> full-plane AP;  (1, c, h) -> half-plane AP
        tf = {}
        for c in range(C):
            tl = chpool.tile([P, FD], mybir.dt.bfloat16, name=f"ch{c}", tag=f"n0ch{c}")
            # SWDGE cast-DMA: f32 HBM -> bf16 SBUF
            nc.gpsimd.dma_start(out=tl, in_=yp[0, c])
            ch[0, c] = tl
        for n in range(NB):
            ti = tpool.tile([P, FD], mybir.dt.int32, name="ti", tag=f"ti{n}")
            nc.sync.dma_start(out=ti, in_=yt[n])
            # labels to bf16 (exact for 0..7) on ScalarE
            tfn = tpool.tile([P, FD], mybir.dt.bfloat16, name="tf", tag=f"tf{n}")
            nc.scalar.copy(out=tfn, in_=ti)
            tf[n] = tfn
        im1 = {}
        for c in range(C):
            im1[c] = chpool.tile([P, FD], mybir.dt.bfloat16, name=f"ch{c}", tag=f"n1ch{c}")
        for h in range(2):
            for c in range(C):
                half = im1[c][:, h * HF : (h + 1) * HF]
                nc.gpsimd.dma_start(out=half, in_=yp[1, c][:, h * HF : (h + 1) * HF])
                ch[1, c, h] = half

        def emit_gt(slot, tfv, c, s0, ns):
            """gt mask (DVE 4x) + gt count (ScalarE flat copy w/ accum).
            Writes subtile blocks s0..s0+ns of class c's gt tile."""
            g = gts[c - 1]
            gv = g[:, s0 : s0 + ns, 0:128]
            nc.vector.tensor_single_scalar(
                out=gv, in_=tfv, scalar=float(c), op=mybir.AluOpType.is_equal
            )
            scr = scrp.tile([P, NSUB * BLK], mybir.dt.bfloat16, name="scr", tag="scr")
            # flat contiguous read (incl. ones + zero pad; host subtracts
            # the constant 16 per partition per block) keeps ScalarE fast
            nc.scalar.activation(
                out=scr[:, 0 : ns * BLK],
                in_=g[:, s0 : s0 + ns, :].rearrange("p s f -> p (s f)"),
                func=mybir.ActivationFunctionType.Copy,
                accum_out=ga_acc[:, slot : slot + 1],
            )

        def emit_tree(chs, fd):
            m01 = mtmp.tile([P, FD], mybir.dt.bfloat16, name="m01", tag="mt")
            nc.vector.tensor_max(m01[:, 0:fd], chs[0], chs[1])
            m23 = mtmp.tile([P, FD], mybir.dt.bfloat16, name="m23", tag="mt")
            nc.vector.tensor_max(m23[:, 0:fd], chs[2], chs[3])
            m0123 = mtmp.tile([P, FD], mybir.dt.bfloat16, name="m0123", tag="mt")
            nc.vector.tensor_max(m0123[:, 0:fd], m01[:, 0:fd], m23[:, 0:fd])
            m45 = mtmp.tile([P, FD], mybir.dt.bfloat16, name="m45", tag="mt")
            nc.vector.tensor_max(m45[:, 0:fd], chs[4], chs[5])
            m67 = mtmp.tile([P, FD], mybir.dt.bfloat16, name="m67", tag="mt")
            nc.vector.tensor_max(m67[:, 0:fd], chs[6], chs[7])
            m4567 = mtmp.tile([P, FD], mybir.dt.bfloat16, name="m4567", tag="mt")
            nc.vector.tensor_max(m4567[:, 0:fd], m45[:, 0:fd], m67[:, 0:fd])
            m = mpool.tile([P, FD], mybir.dt.bfloat16, name="m", tag="m")
            nc.vector.tensor_max(m[:, 0:fd], m0123[:, 0:fd], m4567[:, 0:fd])
            return m

        def emit_pred_mm(chv, m, c, s0, ns, start, stop):
            pred = predp.tile([P, FD], mybir.dt.bfloat16, name=f"pred{c}", tag="pred")
            predv = pred[:, 0 : ns * 128]
            nc.vector.tensor_tensor(
                out=predv, in0=chv, in1=m, op=mybir.AluOpType.is_equal
            )
            g = gts[c - 1]
            for s in range(ns):
                nc.tensor.matmul(
                    psums[c - 1][:, :],
                    lhsT=predv[:, s * 128 : (s + 1) * 128],
                    rhs=g[:, s0 + s, 0:129],
                    start=(start and s == 0),
                    stop=(stop and s == ns - 1),
                )

        # ---- DVE program, ordered to match SWDGE arrival times ----
        # image 0: early tree half, gt masks as arrival-gap filler, late
        # tree, then pred+MM per class.
        m01 = mtmp.tile([P, FD], mybir.dt.bfloat16, name="m01", tag="mt")
        nc.vector.tensor_max(m01, ch[0, 0], ch[0, 1])
        m23 = mtmp.tile([P, FD], mybir.dt.bfloat16, name="m23", tag="mt")
        nc.vector.tensor_max(m23, ch[0, 2], ch[0, 3])
        m0123 = mtmp.tile([P, FD], mybir.dt.bfloat16, name="m0123", tag="mt")
        nc.vector.tensor_max(m0123, m01, m23)
        tf3_0 = tf[0].rearrange("p (s f) -> p s f", s=NSUB)
        for c in range(1, C):
            emit_gt(c - 1, tf3_0, c, 0, NSUB)
        m45 = mtmp.tile([P, FD], mybir.dt.bfloat16, name="m45", tag="mt")
        nc.vector.tensor_max(m45, ch[0, 4], ch[0, 5])
        m67 = mtmp.tile([P, FD], mybir.dt.bfloat16, name="m67", tag="mt")
        nc.vector.tensor_max(m67, ch[0, 6], ch[0, 7])
        m4567 = mtmp.tile([P, FD], mybir.dt.bfloat16, name="m4567", tag="mt")
        nc.vector.tensor_max(m4567, m45, m67)
        m0 = mpool.tile([P, FD], mybir.dt.bfloat16, name="m", tag="m")
        nc.vector.tensor_max(m0, m0123, m4567)
        for c in range(1, C):
            emit_pred_mm(ch[0, c], m0, c, 0, NSUB, start=True, stop=False)

        # image 1, by half-plane: gt masks first (need only labels), then
        # per half: tree + pred/MM.
        tf3_1 = tf[1].rearrange("p (s f) -> p s f", s=NSUB)
        for h in range(2):
            for c in range(1, C):
                emit_gt(7 + h * 7 + (c - 1), tf3_1[:, h * HS : (h + 1) * HS, :], c, h * HS, HS)
        for h in range(2):
            mh = emit_tree([ch[1, c, h] for c in range(C)], HF)
            for c in range(1, C):
                emit_pred_mm(
                    ch[1, c, h], mh[:, 0:HF], c, h * HS, HS,
                    start=False, stop=(h == 1),
                )

        nc.sync.dma_start(out=ga_out[:], in_=ga_acc)
        for c in range(7):
            pt = accp.tile([P, 129], mybir.dt.float32, name=f"pt{c}", tag=f"pt{c}")
            nc.scalar.copy(out=pt, in_=psums[c])
            nc.sync.dma_start(out=mm_out[c], in_=pt)

    nc.finalize()
    return nc


def _get_bass():
    global _CACHED_NC
    if _CACHED_NC is None:
        _CACHED_NC = build_bass()
    return _CACHED_NC


def make_in_maps(y_true, y_pred):
    yp = np.ascontiguousarray(np.asarray(y_pred, dtype=np.float32))
    yt = np.ascontiguousarray(np.asarray(y_true, dtype=np.int32))
    in_maps = []
    for i in range(N_CORES):
        yps = np.ascontiguousarray(yp[NB * i : NB * (i + 1)]).reshape(NB, C, P, FD)
        yts = np.ascontiguousarray(yt[NB * i : NB * (i + 1)]).reshape(NB, P, FD)
        in_maps.append({"yp": yps, "yt": yts})
    return in_maps


def epilogue(results):
    """Combine the 8 cores' partial sums into the final dice mean (float32,
    mirroring the reference arithmetic)."""
    tp = np.zeros(7, dtype=np.float64)
    pred_cnt = np.zeros(7, dtype=np.float64)
    gt_cnt = np.zeros(7, dtype=np.float64)
    for r in results:
        mm = np.asarray(r["mm_out"], dtype=np.float64)  # [7, P, 129]
        tp += np.trace(mm[:, :, :128], axis1=1, axis2=2)
        pred_cnt += mm[:, :, 128].sum(axis=1)
        ga = np.asarray(r["ga_out"], dtype=np.float64).sum(axis=0)  # [21]
        # each slot's flat accum includes one ones-column entry per block
        # per partition: 16 blocks (img0) + 8 + 8 (img1 halves) = 32*P total
        gt_cnt += ga[0:7] + ga[7:14] + ga[14:21] - 2 * 16 * P

    tp32 = tp.astype(np.float32)
    fp32_ = (pred_cnt - tp).astype(np.float32)
    fn32 = (gt_cnt - tp).astype(np.float32)
    eps = np.float32(EPS)
    two = np.float32(2.0)
    dice = (two * tp32 + eps) / (two * tp32 + fp32_ + fn32 + eps)
    return np.asarray(np.mean(dice, dtype=np.float32), dtype=np.float32)


def kernel(**inputs):
    from concourse.bass_utils import run_bass_kernel_spmd

    nc = _get_bass()
    in_maps = make_in_maps(inputs["y_true"], inputs["y_pred"])
    res = run_bass_kernel_spmd(nc, in_maps, core_ids=list(range(N_CORES)))
    return epilogue(res.results)


if __name__ == "__main__":
    # smoke test with random data
    rng = np.random.default_rng(0)
    y_true = rng.integers(0, C, size=(16, 512, 512)).astype(np.int32)
    y_pred = rng.standard_normal((16, C, 512, 512)).astype(np.float32)
    out = kernel(y_true=y_true, y_pred=y_pred)
    print("kernel output:", out)
